# revision 25
# baseline (speedup 1.0000x reference)
"""Trainium2 Bass kernel for nn_MixedRepeatHeads (sparse_attention).

Math (per batch element b, derived from the reference):
  proj[t, hd]  = sum_e x[e, t] * W1[e, hd] + proj_b          (W1 = proj_w^T)
  mixed[s, hd] = c_h[s] * sum_{t<=s} a_h[t] * proj[t, hd] * r_h^(s-t) + mix_b
                 a_h = mix_w[h] for row-repeat heads (h>=4) else 1
                 c_h = mix_w[h] for col-repeat heads (h<4) else 1
                 r_h = clip(decay_v,0.9,1)^(1/4)  (== 1 for the reference seed)
  out[dout, s] = sum_hd out_w[dout, hd] * mixed[s, hd] + out_b[dout]

Device strategy (data-parallel: one batch element per NeuronCore, 8 cores,
no collectives):
  M1: (t,hd)-layout projection matmuls, fp16 in / fp32 PSUM.
  M2: chunked cumulative scan over t via 128x128 triangular matmuls (fp16);
      the row-head premul a_h[t] is folded into per-(head,tile) U matrices;
      per-head carry A[d] enters as a per-partition bias at eviction
      (exact only for decay==1; other decay falls back to numpy on host);
      col-head postmul c_h[s] via partition-replicated crep tile (DVE).
      Output layout (d, s) per head == hd on partitions, ready for M3.
  M3: out = W3^T-tiles @ mixed + rank-17 bias augmentation (proj_b through
      the mixer, mix_b, out_b) as a K=17 matmul; interleaved into phase A
      per 512-column s-block (block n runs after t-tiles 4n..4n+3) to keep
      the PE warm and spread DMA.

All bias terms fold on the host into Phi (17, T) / Psi (17, DIM):
  out_bias[dout, s] = sum_r Psi[r, dout] * Phi[r, s]

fp16 operands give the same 10-bit-mantissa precision class as the PE's
fp32r (TF32-like) mode but run at 1 cycle/row at any moving size and use
fast weight loads; accumulation is always fp32 in PSUM.
"""

import os

import numpy as np

import concourse.mybir as mybir
import concourse.tile as tile
from concourse import bacc
from concourse.bass_utils import run_bass_kernel_spmd

B = 8
E = 1024
T = 2048
H = 8
D = 128
HD = H * D
H2 = H // 2
DECAY_CONST = 4
C = 128          # cumsum chunk (= t-tile = partition size)
NT = T // C      # 16 t-tiles
NE = E // 128    # 8 e-tiles
NM = HD // 128   # 8 dout-tiles
NB = 4           # M3 s-blocks (512 cols each)
KA = 17          # bias rank: 8 (proj_b) + 8 (mix_b) + 1 (out_b)

FP32 = mybir.dt.float32
FP16 = mybir.dt.float16

_module_cache: dict = {}


SKEW = 4  # M2 lags M1 by this many t-tiles (absorbs const-DMA latency)


def _emit(tc, aps, repeat: int = 1, hw_loop: int = 1):
    nc = tc.nc
    xt, w1, w3, ucol, urow, crep, bcorr, out = (
        aps["xt"], aps["w1"], aps["w3"], aps["ucol"], aps["urow"],
        aps["crep"], aps["bcorr"], aps["out"],
    )

    with (
        tc.tile_pool(name="const", bufs=1) as consts,
        tc.tile_pool(name="xt", bufs=SKEW + 2) as xtp,
    ):
        # prefetch the first x tiles before the big consts so M1 starts early
        xt_tiles = {}
        for i in range(2):
            xt_pre = xtp.tile([128, NE * 128], FP16, tag="xt", name=f"xt_pre{i}")
            if i == 0:
                nc.sync.dma_start(xt_pre[:, 0:256], xt[i, :, 0:256])
                nc.sync.dma_start(xt_pre[:, 256:], xt[i, :, 256:])
            else:
                nc.sync.dma_start(xt_pre[:], xt[i, :, :])
            xt_tiles[i] = xt_pre

        w1_sb = consts.tile([128, NE * HD], FP16, tag="w1")        # 2 MB
        nc.sync.dma_start(w1_sb[:, 0:HD], w1[:, 0:HD])   # j=0 slice first
        for lo, hi in ((1, 3), (3, 5), (5, 8)):
            nc.sync.dma_start(w1_sb[:, lo * HD:hi * HD],
                              w1[:, lo * HD:hi * HD])
        ucol_sb = consts.tile([128, H2 * C], FP16, tag="ucol")
        nc.sync.dma_start(ucol_sb[:], ucol[:])
        # urow/crep are tile-major: chunk g covers t-tiles 4g..4g+3; chunk 0
        # is emitted here, the rest stream from inside the loop
        urow_sb = consts.tile([128, NT * H2 * C], FP16, tag="urow")  # 2 MB
        crep_sb = consts.tile([128, NT * H2 * C], FP32, tag="crep")  # 4 MB
        w3_sb = consts.tile([128, NM * NM * 128], FP16, tag="w3")  # 2 MB
        uq = NT * H2 * C // 4
        nc.sync.dma_start(urow_sb[:, 0:uq], urow[:, 0:uq])
        nc.sync.dma_start(crep_sb[:, 0:uq], crep[:, 0:uq])

        def emit_late_consts(g):
            # called per 4-tile group from inside the pass loop (first rep)
            if g >= 1:
                nc.sync.dma_start(urow_sb[:, g * uq:(g + 1) * uq],
                                  urow[:, g * uq:(g + 1) * uq])
                nc.sync.dma_start(crep_sb[:, g * uq:(g + 1) * uq],
                                  crep[:, g * uq:(g + 1) * uq])
            if g == 0:
                nc.sync.dma_start(w3_sb[:], w3[:])

        with (
            tc.tile_pool(name="mixed", bufs=1) as mixedp,
            tc.tile_pool(name="projS", bufs=SKEW + 2) as projp,
            tc.tile_pool(name="acarry", bufs=3) as acp,
            tc.tile_pool(name="outS", bufs=3) as outp,
            tc.tile_pool(name="psum1", bufs=2, space="PSUM") as ps1p,
            tc.tile_pool(name="pssm", bufs=4, space="PSUM") as pssm,
            tc.tile_pool(name="bcorr", bufs=16) as bcp,
        ):
            pools = (mixedp, projp, acp, outp, ps1p, pssm, bcp)
            if hw_loop > 1:
                # load remaining consts up-front, then time a hardware loop
                for g in range(4):
                    emit_late_consts(g if g else 0)
                with tc.For_i(0, hw_loop, 1):
                    _emit_one_pass(tc, nc, xt, out, bcorr, w1_sb, w3_sb,
                                   ucol_sb, urow_sb, crep_sb, pools,
                                   xtp, {}, lambda g: None)
            else:
                for rep in range(repeat):
                    _emit_one_pass(tc, nc, xt, out, bcorr, w1_sb, w3_sb,
                                   ucol_sb, urow_sb, crep_sb, pools,
                                   xtp, xt_tiles if rep == 0 else {},
                                   emit_late_consts if rep == 0
                                   else lambda g: None)


def _emit_one_pass(tc, nc, xt, out, bcorr, w1_sb, w3_sb, ucol_sb, urow_sb,
                   crep_sb, pools, xtp, xt_tiles, emit_late_consts):
    Ident = mybir.ActivationFunctionType.Identity
    mixedp, projp, acp, outp, ps1p, pssm, bcp = pools
    mixed_sb = mixedp.tile([128, H * T], FP16, tag="mixed")        # 4 MB
    if True:
        a_prev = None
        projS_t = {}

        def emit_m1(i):
            if i in xt_tiles:
                xt_i = xt_tiles[i]
            else:
                xt_i = xtp.tile([128, NE * 128], FP16, tag="xt")
                nc.sync.dma_start(xt_i[:], xt[i, :, :])
            psum1 = ps1p.tile([128, HD], FP32, tag="psum1")
            for n in range(2):
                for j in range(NE):
                    nc.tensor.matmul(
                        psum1[:, n * 512:(n + 1) * 512],
                        xt_i[:, j * 128:(j + 1) * 128],
                        w1_sb[:, j * HD + n * 512: j * HD + (n + 1) * 512],
                        start=(j == 0), stop=(j == NE - 1),
                    )
            projS = projp.tile([128, HD], FP16, tag="projS")
            nc.scalar.copy(projS[:], psum1[:])
            projS_t[i] = projS

        def emit_m2(i, last=False):
            nonlocal a_prev
            projS = projS_t.pop(i)
            ps2 = [pssm.tile([128, 512], FP32, tag="pssm", name=f"ps2_{i}_{g}")
                   for g in range(2)]
            for h in range(H):
                if h < H2:
                    u_slice = ucol_sb[:, h * C:(h + 1) * C]
                else:
                    u_slice = urow_sb[:, (i * H2 + h - H2) * C:
                                      (i * H2 + h - H2 + 1) * C]
                nc.tensor.matmul(
                    ps2[h // 4][:, (h % 4) * D:(h % 4 + 1) * D],
                    projS[:, h * D:(h + 1) * D],
                    u_slice,
                    start=True, stop=True,
                )
            # carry update: A_next[d, h] = psum2[d, last col of h] + A_prev
            a_next = None
            if not last:
                a_next = acp.tile([128, H], FP32, tag="acarry")
                for g in range(2):
                    src = ps2[g][:, C - 1::C]
                    dstA = a_next[:, g * 4:(g + 1) * 4]
                    if a_prev is None:
                        nc.vector.tensor_copy(dstA, src)
                    else:
                        nc.vector.tensor_tensor(
                            dstA, src, a_prev[:, g * 4:(g + 1) * 4],
                            op=mybir.AluOpType.add)
            # evictions into mixed (fp16)
            for h in range(H):
                dst = mixed_sb[:, h * T + i * C: h * T + (i + 1) * C]
                src = ps2[h // 4][:, (h % 4) * D:(h % 4 + 1) * D]
                if h < H2:
                    crep_slice = crep_sb[:, (i * H2 + h) * C:
                                         (i * H2 + h + 1) * C]
                    # (psum + A) * c  in one DVE op
                    if a_prev is None:
                        nc.vector.tensor_tensor(
                            dst, src, crep_slice, op=mybir.AluOpType.mult)
                    else:
                        nc.vector.scalar_tensor_tensor(
                            dst, src, a_prev[:, h:h + 1], crep_slice,
                            op0=mybir.AluOpType.add,
                            op1=mybir.AluOpType.mult)
                else:
                    if a_prev is None:
                        nc.scalar.copy(dst, src)
                    else:
                        nc.scalar.activation(
                            dst, src, Ident, bias=a_prev[:, h:h + 1])
            a_prev = a_next

        bc_tiles = {}

        def prefetch_bcorr(col0, width):
            ns = slice(col0, col0 + width)
            for m in range(NM):
                bc_t = bcp.tile([128, 512], FP32, tag="bcorr",
                                name=f"bc_{col0}_{m}")
                nc.sync.dma_start(bc_t[:, 0:width],
                                  bcorr[m * 128:(m + 1) * 128, ns])
                bc_tiles[(col0, m)] = bc_t

        def emit_m3_block(col0, width):
            ns = slice(col0, col0 + width)
            if (col0, 0) not in bc_tiles:
                prefetch_bcorr(col0, width)
            for m in range(NM):
                bc_t = bc_tiles.pop((col0, m))
                psum3 = pssm.tile([128, 512], FP32, tag="pssm")
                for k in range(NM):
                    nc.tensor.matmul(
                        psum3[:, 0:width],
                        w3_sb[:, m * HD + k * 128: m * HD + (k + 1) * 128],
                        mixed_sb[:, k * T + col0: k * T + col0 + width],
                        start=(k == 0), stop=(k == NM - 1),
                    )
                outS = outp.tile([128, 512], FP32, tag="outS")
                nc.vector.tensor_tensor(
                    outS[:, 0:width], psum3[:, 0:width], bc_t[:, 0:width],
                    op=mybir.AluOpType.add)
                nc.sync.dma_start(out[m * 128:(m + 1) * 128, ns],
                                  outS[:, 0:width])

        for i in range(NT + SKEW):
            if i < NT:
                emit_m1(i)
                if i == 3:
                    emit_late_consts(0)
                if i in (6, 9, 12):
                    emit_late_consts((i - 3) // 3)
            if i >= SKEW:
                i2 = i - SKEW
                emit_m2(i2, last=(i2 == NT - 1))
                if i2 in (4, 8, 12):
                    emit_m3_block((i2 // 4 - 1) * 512, 512)
                elif i2 == 14:
                    emit_m3_block(1536, 256)
                    prefetch_bcorr(1792, 256)
                elif i2 == 15:
                    emit_m3_block(1792, 256)
                if i2 in (2, 6, 10):
                    prefetch_bcorr(((i2 - 2) // 4) * 512, 512)
                elif i2 == 12:
                    prefetch_bcorr(1536, 256)


def _build_module(repeat: int = 1, hw_loop: int = 1):
    key = ("v7", repeat, hw_loop)
    if key in _module_cache:
        return _module_cache[key]
    nc = bacc.Bacc("TRN2", target_bir_lowering=False, debug=False,
                   enable_asserts=False)
    aps = {
        "xt": nc.dram_tensor("xt", [NT, 128, NE * 128], FP16,
                             kind="ExternalInput").ap(),
        "w1": nc.dram_tensor("w1", [128, NE * HD], FP16,
                             kind="ExternalInput").ap(),
        "w3": nc.dram_tensor("w3", [128, NM * NM * 128], FP16,
                             kind="ExternalInput").ap(),
        "ucol": nc.dram_tensor("ucol", [128, H2 * C], FP16,
                               kind="ExternalInput").ap(),
        "urow": nc.dram_tensor("urow", [128, NT * H2 * C], FP16,
                               kind="ExternalInput").ap(),
        "crep": nc.dram_tensor("crep", [128, NT * H2 * C], FP32,
                               kind="ExternalInput").ap(),
        "bcorr": nc.dram_tensor("bcorr", [HD, T], FP32,
                                kind="ExternalInput").ap(),
        "out": nc.dram_tensor("out", [HD, T], FP32,
                              kind="ExternalOutput").ap(),
    }
    with tile.TileContext(nc) as tc:
        _emit(tc, aps, repeat=repeat, hw_loop=hw_loop)
    nc.compile()
    _module_cache[key] = (nc, aps)
    return nc, aps


def _host_prep(x, proj_w, proj_b, mix_w, mix_b, decay_v, out_w, out_b):
    """Build per-core input maps (numpy only)."""
    x = np.ascontiguousarray(np.asarray(x, dtype=np.float32))
    proj_w = np.asarray(proj_w, dtype=np.float32)
    proj_b = np.asarray(proj_b, dtype=np.float32)
    mix_w = np.asarray(mix_w, dtype=np.float32)
    mix_b = np.asarray(mix_b, dtype=np.float32)
    decay_v = np.asarray(decay_v, dtype=np.float32)
    out_w = np.asarray(out_w, dtype=np.float32)
    out_b = np.asarray(out_b, dtype=np.float32)

    dclip = np.clip(decay_v, 0.9, 1.0)
    r = (dclip.astype(np.float64)) ** (1.0 / DECAY_CONST)

    a = np.ones((H, T), np.float32)
    a[H2:] = mix_w[H2:]
    c = np.ones((H, T), np.float32)
    c[:H2] = mix_w[:H2]

    tau = np.arange(C)[:, None]
    sig = np.arange(C)[None, :]
    U = np.where(sig >= tau,
                 r[:, None, None] ** (sig - tau).astype(np.float64),
                 0.0).astype(np.float32)                    # (H, C, C)
    ucol = np.concatenate([U[h] for h in range(H2)], axis=1)  # (128, H2*C)
    # row heads: premul a_h[tau] folded in; tile-major [(i*H2 + hr)*C]
    urow = np.zeros((128, NT * H2 * C), np.float32)
    for i in range(NT):
        for hr in range(H2):
            h = H2 + hr
            blk = U[h] * a[h, i * C:(i + 1) * C][:, None]    # (tau, sigma)
            urow[:, (i * H2 + hr) * C:(i * H2 + hr + 1) * C] = blk

    # col-head postmul values, partition-replicated, tile-major [(i*H2+h)*C]
    crep = np.zeros((128, NT * H2 * C), np.float32)
    for i in range(NT):
        for h in range(H2):
            crep[:, (i * H2 + h) * C:(i * H2 + h + 1) * C] = \
                c[h, i * C:(i + 1) * C][None, :]

    g = np.zeros((H, T), np.float64)
    for h in range(H):
        acc = 0.0
        for s in range(T):
            acc = acc * r[h] + float(a[h, s])
            g[h, s] = acc
    Phi = np.concatenate(
        [(c.astype(np.float64) * g).astype(np.float32),
         mix_b, np.ones((1, T), np.float32)], axis=0)        # (17, T)
    psi1 = np.stack([out_w[:, h * D:(h + 1) * D] @ proj_b[h]
                     for h in range(H)], axis=0)
    psi2 = np.stack([out_w[:, h * D:(h + 1) * D].sum(1)
                     for h in range(H)], axis=0)
    Psi = np.concatenate([psi1, psi2, out_b[None, :]], axis=0).astype(np.float32)
    Bcorr = np.ascontiguousarray(
        (Psi.astype(np.float64).T @ Phi.astype(np.float64))
        .astype(np.float32))                                  # (DIM, T)

    W1 = np.ascontiguousarray(proj_w.transpose(2, 0, 1).reshape(E, HD))
    w1_host = np.ascontiguousarray(
        W1.reshape(NE, 128, HD).transpose(1, 0, 2).reshape(128, NE * HD))

    W3 = np.ascontiguousarray(out_w.T)                       # (hd, dout)
    # w3_host[p, m*HD + k*128 + c] = W3[k*128+p, m*128+c]
    w3_host = np.ascontiguousarray(
        W3.reshape(NM, 128, NM, 128).transpose(1, 2, 0, 3)
        .reshape(128, NM * NM * 128))

    f16 = np.float16
    shared = {
        "w1": w1_host.astype(f16), "w3": w3_host.astype(f16),
        "ucol": ucol.astype(f16), "urow": urow.astype(f16),
        "crep": crep, "bcorr": Bcorr,
    }

    in_maps = []
    for b in range(B):
        xt = np.ascontiguousarray(
            x[b].reshape(NE, 128, NT, C).transpose(2, 1, 0, 3)
            .reshape(NT, 128, NE * 128).astype(f16))
        m = {"xt": xt}
        m.update(shared)
        in_maps.append(m)
    return in_maps


def _numpy_fallback(x, proj_w, proj_b, mix_w, mix_b, decay_v, out_w, out_b):
    """Exact reference math in numpy (used only if decay_v != 1)."""
    x = np.asarray(x, np.float32)
    S = T
    i = np.arange(S)[:, None]
    j = np.arange(S)[None, :]
    mask = j >= i
    expo = np.where(mask, (j - i) / DECAY_CONST, 0.0).astype(np.float32)
    d = np.clip(np.asarray(decay_v, np.float32), 0.9, 1.0)
    dpow = d[:, None, None] ** expo[None]
    col_v = np.broadcast_to(np.asarray(mix_w)[:H2, None, :], (H2, S, S))
    row_v = np.broadcast_to(np.asarray(mix_w)[H2:, :, None], (H - H2, S, S))
    vmat = np.concatenate([col_v, row_v], axis=0)
    M = np.where(mask[None], vmat * dpow, 0.0).astype(np.float32)
    x_bte = x.transpose(0, 2, 1)
    proj = np.einsum('bte,hde->bhtd', x_bte, np.asarray(proj_w, np.float32)) \
        + np.asarray(proj_b, np.float32)[None, :, None, :]
    mixed = np.einsum('bhtd,hts->bhsd', proj, M) \
        + np.asarray(mix_b, np.float32)[None, :, :, None]
    Bn, Hn, Sn, Dn = mixed.shape
    hidden = mixed.transpose(0, 2, 1, 3).reshape(Bn, Sn, Hn * Dn)
    outv = hidden @ np.asarray(out_w, np.float32).T + np.asarray(out_b, np.float32)
    return outv.transpose(0, 2, 1).astype(np.float32)


def kernel(**inputs) -> np.ndarray:
    decay_v = np.asarray(inputs["decay_v"], np.float32)
    if not np.all(np.clip(decay_v, 0.9, 1.0) == 1.0):
        return _numpy_fallback(**inputs)

    in_maps = _host_prep(**inputs)
    repeat = int(os.environ.get("KERNEL_REPEAT", "1"))
    nc, _aps = _build_module(repeat=repeat)
    res = run_bass_kernel_spmd(nc, in_maps, core_ids=list(range(B)))
    out = np.stack([res.results[b]["out"] for b in range(B)], axis=0)
    return out.astype(np.float32)


if __name__ == "__main__":
    rng = np.random.default_rng(0)
    demo = {
        "x": rng.standard_normal((B, E, T), dtype=np.float32),
        "proj_w": rng.standard_normal((H, D, E), dtype=np.float32) / 32,
        "proj_b": rng.standard_normal((H, D), dtype=np.float32) * 0.01,
        "mix_w": rng.standard_normal((H, T), dtype=np.float32),
        "mix_b": np.zeros((H, T), np.float32),
        "decay_v": np.ones((H,), np.float32),
        "out_w": rng.standard_normal((E, E), dtype=np.float32) / 32,
        "out_b": rng.standard_normal((E,), dtype=np.float32) * 0.01,
    }
    got = kernel(**demo)
    exp = _numpy_fallback(**demo)
    err = np.abs(got - exp).max()
    print("absmax err vs numpy:", err, "rel:", err / np.abs(exp).max())
